# revision 1
# baseline (speedup 1.0000x reference)
"""CausalWanSelfAttention Trainium2 kernel — 8-core SPMD, 3 phases.

Phase 1 (row-sharded, 195 rows/core): q/k/v projections (full 1536 cols,
  needed because rms_norm couples all heads), rms-norm, RoPE.
Phase 2 (half-head sharded, 3 units/core): attention with transposed scores
  [keys, q] so softmax-divide folds into a ones-matmul denominator; no
  max-subtraction (scores ~ N(0,1) by construction).
Phase 3 (row-sharded): output projection + bias.

Host code between phases only reshapes / transposes / concatenates.
"""

import numpy as np

import concourse.bass as bass
import concourse.bacc as bacc
import concourse.mybir as mybir
import concourse.tile as tile
from concourse.bass_utils import run_bass_kernel_spmd

F32 = mybir.dt.float32
F32R = mybir.dt.float32r

N_CORES = 8
DIM = 1536
NH = 12
HD = 128
S = 1560
R = S // N_CORES            # 195 rows per core (phases 1/3)
CUR_START = 4680
WIN = CUR_START + S         # 6240 attended keys
KB = 128                    # key block (partition) size in phase 2
N_KB = (WIN + KB - 1) // KB  # 49 (last block 96 keys)
LAST_KP = WIN - (N_KB - 1) * KB  # 96
HH = S // 2                 # 780 queries per half-head unit
QC = HH // 2                # 390-query chunks (>=256 keeps f32r fast)
UNITS = 3                   # half-head units per core (24 units / 8 cores)
EPS = 1e-6
SCALE = float(1.0 / np.sqrt(HD))
HALF_PI = float(np.pi / 2)

USE_F32R = True             # flip to False for full-precision matmuls
MM_DT = F32R if USE_F32R else F32   # dtype of every matmul-feeding tensor

_programs = {}


def _bcast_rows(handle, n, rows=128):
    """AP reading a [n] DRAM tensor broadcast across `rows` partitions."""
    return bass.AP(tensor=handle, offset=0, ap=[[0, rows], [1, n]])


def _swap_pairs(ap_2d, rows, heads=NH, pairs=HD // 2):
    """View of [rows, heads*pairs*2] with each (even,odd) pair swapped."""
    p_step = ap_2d.ap[0][0]
    return bass.AP(
        tensor=ap_2d.tensor,
        offset=ap_2d.offset + 1,
        ap=[[p_step, rows], [2 * pairs, heads], [2, pairs], [-1, 2]],
    )


def _head_bcast(ap_2d, rows, heads=NH, width=HD):
    """View of a [rows, width] tile broadcast to [rows, heads, width]."""
    p_step = ap_2d.ap[0][0]
    return bass.AP(
        tensor=ap_2d.tensor,
        offset=ap_2d.offset,
        ap=[[p_step, rows], [0, heads], [1, width]],
    )


# --------------------------------------------------------------------------
# Phase 1: x[rows] -> q_roped, k_roped, v   (all [R, DIM])
# --------------------------------------------------------------------------
def _build_phase1():
    nc = bacc.Bacc()
    xT = nc.dram_tensor("xT", [NH, 128, R], MM_DT, kind="ExternalInput")
    wT = {}
    b_in = {}
    for w in ("q", "k", "v"):
        wT[w] = nc.dram_tensor(f"w{w}T", [NH, 128, DIM], MM_DT, kind="ExternalInput")
        b_in[w] = nc.dram_tensor(f"b{w}", [DIM], F32, kind="ExternalInput")
    g_in = {w: nc.dram_tensor(f"g{w}", [DIM], F32, kind="ExternalInput")
            for w in ("q", "k")}
    ang_in = nc.dram_tensor("ang", [R, HD // 2], F32, kind="ExternalInput")
    outs = {w: nc.dram_tensor(f"{w}_out", [R, DIM], F32, kind="ExternalOutput")
            for w in ("q", "k", "v")}

    row_blocks = [(0, 128), (128, R - 128)]

    with tile.TileContext(nc) as tc:
        with (
            tc.tile_pool(name="consts", bufs=1) as consts,
            tc.tile_pool(name="wstream", bufs=4) as wstream,
            tc.tile_pool(name="acts", bufs=1) as acts,
            tc.tile_pool(name="scratch", bufs=2) as scratch,
            tc.tile_pool(name="rope", bufs=2) as rope,
            tc.tile_pool(name="small", bufs=4) as small,
            tc.tile_pool(name="psum", bufs=1, space="PSUM") as psum,
        ):
            negpi_t = consts.tile([128, 1], F32, tag="negpi")
            nc.vector.memset(negpi_t, -float(np.pi))
            eps_t = consts.tile([128, 1], F32, tag="epsc")
            nc.vector.memset(eps_t, EPS)
            xt = consts.tile([128, NH, R], MM_DT, tag="xT")
            nc.sync.dma_start(out=xt, in_=xT.ap().rearrange("k p r -> p k r"))
            bias_t = {}
            for w in ("q", "k", "v"):
                bias_t[w] = consts.tile([128, DIM], F32, tag=f"b{w}", name=f"b{w}t")
                nc.gpsimd.dma_start(out=bias_t[w], in_=_bcast_rows(b_in[w], DIM))
            g_t = {}
            for w in ("q", "k"):
                g_t[w] = consts.tile([128, DIM], F32, tag=f"g{w}", name=f"g{w}t")
                nc.gpsimd.dma_start(out=g_t[w], in_=_bcast_rows(g_in[w], DIM))

            sb = {}
            for w in ("q", "k", "v"):
                ps = {}
                for rb in range(2):
                    for ci in range(3):
                        ps[rb, ci] = psum.tile([128, 512], F32,
                                               tag=f"ps{rb}{ci}",
                                               name=f"ps{rb}{ci}")
                for kt in range(NH):
                    wtile = wstream.tile([128, DIM], MM_DT, tag="wchunk")
                    nc.sync.dma_start(out=wtile, in_=wT[w][kt])
                    for rb, (r0, rows) in enumerate(row_blocks):
                        for ci in range(3):
                            nc.tensor.matmul(
                                ps[rb, ci][:rows],
                                xt[:, kt, r0:r0 + rows],
                                wtile[:, ci * 512:(ci + 1) * 512],
                                start=(kt == 0),
                                stop=(kt == NH - 1),
                            )
                for rb, (r0, rows) in enumerate(row_blocks):
                    sb[w, rb] = acts.tile([128, DIM], F32, tag=f"sb{w}{rb}",
                                          name=f"sb{w}{rb}")
                    for ci in range(3):
                        nc.vector.tensor_add(
                            sb[w, rb][:rows, ci * 512:(ci + 1) * 512],
                            ps[rb, ci][:rows],
                            bias_t[w][:rows, ci * 512:(ci + 1) * 512],
                        )

            for rb, (r0, rows) in enumerate(row_blocks):
                # ---- v goes straight out ----
                nc.gpsimd.dma_start(out=outs["v"][r0:r0 + rows], in_=sb["v", rb][:rows])

                # ---- rope tables for this row block ----
                ang_t = rope.tile([128, HD // 2], F32, tag="ang")
                nc.gpsimd.dma_start(out=ang_t[:rows], in_=ang_in[r0:r0 + rows])
                csb = rope.tile([128, HD // 2], F32, tag="csb")
                ssb = rope.tile([128, HD // 2], F32, tag="ssb")
                thc = rope.tile([128, HD // 2], F32, tag="thc")
                nc.vector.tensor_scalar_add(thc[:rows], ang_t[:rows], float(np.pi / 2))
                mc = rope.tile([128, HD // 2], F32, tag="mc")
                nc.vector.tensor_scalar(out=mc[:rows], in0=thc[:rows],
                                        scalar1=float(np.pi), scalar2=float(2 * np.pi),
                                        op0=mybir.AluOpType.is_ge, op1=mybir.AluOpType.mult)
                nc.vector.tensor_sub(thc[:rows], thc[:rows], mc[:rows])
                ths = rope.tile([128, HD // 2], F32, tag="ths")
                ms = rope.tile([128, HD // 2], F32, tag="ms")
                nc.vector.tensor_scalar(out=ms[:rows], in0=ang_t[:rows],
                                        scalar1=float(np.pi), scalar2=float(2 * np.pi),
                                        op0=mybir.AluOpType.is_ge, op1=mybir.AluOpType.mult)
                nc.vector.tensor_sub(ths[:rows], ang_t[:rows], ms[:rows])
                nc.scalar.activation(csb[:rows], thc[:rows],
                                     mybir.ActivationFunctionType.Sin)
                nc.scalar.activation(ssb[:rows], ths[:rows],
                                     mybir.ActivationFunctionType.Sin)
                cos2 = rope.tile([128, HD // 2, 2], F32, tag="cos2")
                sinp = rope.tile([128, HD // 2, 2], F32, tag="sinp")
                nc.vector.tensor_copy(cos2[:rows, :, 0], csb[:rows])
                nc.vector.tensor_copy(cos2[:rows, :, 1], csb[:rows])
                nc.scalar.mul(sinp[:rows, :, 0], ssb[:rows], -1.0)
                nc.vector.tensor_copy(sinp[:rows, :, 1], ssb[:rows])
                cos2f = cos2.rearrange("p c t -> p (c t)")
                sinpf = sinp.rearrange("p c t -> p (c t)")

                # ---- norm + rope for q and k ----
                for w in ("q", "k"):
                    scr = scratch.tile([128, DIM], F32, tag="scr")
                    ssq = small.tile([128, 1], F32, tag="ssq")
                    nc.scalar.activation(scr[:rows], sb[w, rb][:rows],
                                         mybir.ActivationFunctionType.Square,
                                         accum_out=ssq[:rows])
                    sd = small.tile([128, 1], F32, tag="sd")
                    nc.scalar.activation(sd[:rows], ssq[:rows],
                                         mybir.ActivationFunctionType.Sqrt,
                                         scale=1.0 / DIM, bias=eps_t[:rows])
                    rstd = small.tile([128, 1], F32, tag="rstd")
                    nc.vector.reciprocal(rstd[:rows], sd[:rows])

                    tg = scratch.tile([128, DIM], F32, tag="tg")
                    nc.vector.tensor_mul(tg[:rows], sb[w, rb][:rows], g_t[w][:rows])
                    t1 = scratch.tile([128, DIM], F32, tag="t1")
                    nc.vector.tensor_mul(
                        t1[:rows].rearrange("p (h x) -> p h x", h=NH),
                        tg[:rows].rearrange("p (h x) -> p h x", h=NH),
                        _head_bcast(cos2f, rows),
                    )
                    t2 = scratch.tile([128, DIM], F32, tag="t2")
                    nc.vector.tensor_mul(
                        t2[:rows].rearrange("p (h x) -> p h x", h=NH),
                        _swap_pairs(tg, rows),
                        _head_bcast(sinpf, rows),
                    )
                    rot = scratch.tile([128, DIM], F32, tag="rot")
                    nc.vector.tensor_add(rot[:rows], t1[:rows], t2[:rows])
                    nc.vector.tensor_scalar_mul(rot[:rows], rot[:rows],
                                                rstd[:rows])
                    nc.gpsimd.dma_start(out=outs[w][r0:r0 + rows],
                                        in_=rot[:rows])
    nc.finalize()
    return nc


# --------------------------------------------------------------------------
# Phase 2: attention for 3 half-head units per core
#   kT [3,128,WIN], v [3,N_KB*128,128] (padded), qT [3,128,HH]
#   -> aoT [3,128,HH]  (attn_out^T, already softmax-normalized)
# --------------------------------------------------------------------------
def _build_phase2():
    nc = bacc.Bacc()
    kT_in = nc.dram_tensor("kT", [UNITS, 128, WIN], MM_DT, kind="ExternalInput")
    v_in = nc.dram_tensor("v", [UNITS, N_KB * KB, HD], MM_DT, kind="ExternalInput")
    qT_in = nc.dram_tensor("qT", [UNITS, 128, HH], MM_DT, kind="ExternalInput")
    ones_in = nc.dram_tensor("ones", [128, 1], MM_DT, kind="ExternalInput")
    ao_out = nc.dram_tensor("aoT", [UNITS, 128, HH], F32, kind="ExternalOutput")

    with tile.TileContext(nc) as tc:
        with (
            tc.tile_pool(name="kv", bufs=2) as kv,
            tc.tile_pool(name="qp", bufs=2) as qp,
            tc.tile_pool(name="ep", bufs=4) as ep,
            tc.tile_pool(name="aop", bufs=2) as aop,
            tc.tile_pool(name="consts", bufs=1) as consts,
            tc.tile_pool(name="ps_s", bufs=3, space="PSUM") as ps_s,
            tc.tile_pool(name="ps_o", bufs=2, space="PSUM") as ps_o,
            tc.tile_pool(name="ps_d", bufs=2, space="PSUM") as ps_d,
        ):
            ones_t = consts.tile([128, 1], MM_DT, tag="ones")
            nc.gpsimd.dma_start(out=ones_t, in_=ones_in.ap())

            for u in range(UNITS):
                kt = kv.tile([128, WIN], MM_DT, tag="kT")
                nc.gpsimd.dma_start(out=kt, in_=kT_in[u])
                vt = kv.tile([128, N_KB, HD], MM_DT, tag="v")
                nc.gpsimd.dma_start(
                    out=vt, in_=v_in[u].rearrange("(b p) d -> p b d", p=KB))
                qt = qp.tile([128, HH], MM_DT, tag="qT")
                nc.gpsimd.dma_start(out=qt, in_=qT_in[u])

                for qc in range(HH // QC):
                    q_sl = qt[:, qc * QC:(qc + 1) * QC]
                    po = ps_o.tile([128, QC], F32, tag="po")
                    pd = ps_d.tile([1, QC], F32, tag="pd")
                    for kb in range(N_KB):
                        kp = KB if kb < N_KB - 1 else LAST_KP
                        ps = ps_s.tile([128, QC], F32, tag="ps")
                        nc.tensor.matmul(
                            ps[:kp],
                            kt[:, kb * KB:kb * KB + kp],
                            q_sl,
                            start=True, stop=True,
                        )
                        et = ep.tile([128, QC], MM_DT, tag="e")
                        nc.scalar.activation(et[:kp], ps[:kp],
                                             mybir.ActivationFunctionType.Exp,
                                             scale=SCALE)
                        nc.tensor.matmul(
                            pd, ones_t[:kp], et[:kp],
                            start=(kb == 0), stop=(kb == N_KB - 1),
                            skip_group_check=True,
                        )
                        nc.tensor.matmul(
                            po, vt[:kp, kb], et[:kp],
                            start=(kb == 0), stop=(kb == N_KB - 1),
                            skip_group_check=True,
                        )
                    rden = aop.tile([1, QC], F32, tag="rden")
                    nc.vector.reciprocal(rden, pd)
                    rden_bc = aop.tile([128, QC], F32, tag="rden_bc")
                    nc.gpsimd.partition_broadcast(rden_bc, rden)
                    ao = aop.tile([128, QC], F32, tag="ao")
                    nc.vector.tensor_mul(ao, po, rden_bc)
                    nc.gpsimd.dma_start(
                        out=ao_out[u][:, qc * QC:(qc + 1) * QC], in_=ao)
    nc.finalize()
    return nc


# --------------------------------------------------------------------------
# Phase 3: out[rows] = attn_out[rows] @ wo.T + bo
#   aT [NH,128,R] (attn_out^T k-tiled), woT [NH,128,DIM]
# --------------------------------------------------------------------------
def _build_phase3():
    nc = bacc.Bacc()
    aT_in = nc.dram_tensor("aT", [NH, 128, R], MM_DT, kind="ExternalInput")
    woT_in = nc.dram_tensor("woT", [NH, 128, DIM], MM_DT, kind="ExternalInput")
    bo_in = nc.dram_tensor("bo", [DIM], F32, kind="ExternalInput")
    out = nc.dram_tensor("out", [R, DIM], F32, kind="ExternalOutput")

    row_blocks = [(0, 128), (128, R - 128)]

    with tile.TileContext(nc) as tc:
        with (
            tc.tile_pool(name="consts", bufs=1) as consts,
            tc.tile_pool(name="wstream", bufs=4) as wstream,
            tc.tile_pool(name="acts", bufs=2) as acts,
            tc.tile_pool(name="psum", bufs=1, space="PSUM") as psum,
        ):
            at = consts.tile([128, NH, R], MM_DT, tag="aT")
            nc.sync.dma_start(out=at, in_=aT_in.ap().rearrange("k p r -> p k r"))
            bo_t = consts.tile([128, DIM], F32, tag="bo")
            nc.gpsimd.dma_start(out=bo_t, in_=_bcast_rows(bo_in, DIM))

            ps = {}
            for rb in range(2):
                for ci in range(3):
                    ps[rb, ci] = psum.tile([128, 512], F32, tag=f"ps{rb}{ci}",
                                           name=f"ps{rb}{ci}")
            for kt in range(NH):
                wtile = wstream.tile([128, DIM], MM_DT, tag="wchunk")
                nc.sync.dma_start(out=wtile, in_=woT_in[kt])
                for rb, (r0, rows) in enumerate(row_blocks):
                    for ci in range(3):
                        nc.tensor.matmul(
                            ps[rb, ci][:rows],
                            at[:, kt, r0:r0 + rows],
                            wtile[:, ci * 512:(ci + 1) * 512],
                            start=(kt == 0),
                            stop=(kt == NH - 1),
                        )
            for rb, (r0, rows) in enumerate(row_blocks):
                osb = acts.tile([128, DIM], F32, tag=f"osb{rb}", name=f"osb{rb}")
                for ci in range(3):
                    nc.vector.tensor_add(
                        osb[:rows, ci * 512:(ci + 1) * 512],
                        ps[rb, ci][:rows],
                        bo_t[:rows, ci * 512:(ci + 1) * 512],
                    )
                nc.gpsimd.dma_start(out=out[r0:r0 + rows], in_=osb[:rows])
    nc.finalize()
    return nc


def _get_program(name):
    if name not in _programs:
        _programs[name] = {"p1": _build_phase1, "p2": _build_phase2,
                           "p3": _build_phase3}[name]()
    return _programs[name]


# --------------------------------------------------------------------------
# Host orchestration
# --------------------------------------------------------------------------
def _build_angles(freqs, grid, start_frame):
    """[S, 64] per-position rope angles — pure indexing of `freqs`."""
    F_, H_, W_ = grid
    c = HD // 2          # 64
    c3 = c // 3          # 21
    f_ang = freqs[start_frame:start_frame + F_, : c - 2 * c3]   # [F, 22]
    h_ang = freqs[:H_, c - 2 * c3: c - c3]                      # [H, 21]
    w_ang = freqs[:W_, c - c3:]                                 # [W, 21]
    ang = np.concatenate([
        np.broadcast_to(f_ang[:, None, None, :], (F_, H_, W_, c - 2 * c3)),
        np.broadcast_to(h_ang[None, :, None, :], (F_, H_, W_, c3)),
        np.broadcast_to(w_ang[None, None, :, :], (F_, H_, W_, c3)),
    ], axis=-1).reshape(F_ * H_ * W_, c)
    return np.ascontiguousarray(ang, dtype=np.float32)


def _run(nc, in_maps, **kw):
    res = run_bass_kernel_spmd(nc, in_maps, core_ids=list(range(N_CORES)), **kw)
    return res


def kernel(x, freqs, wq, bq, wk, bk, wv, bv, wo, bo, gq, gk,
           kv_cache_k, kv_cache_v, grid_sizes, seq_lens, current_start,
           _timings=None):
    x = np.asarray(x, dtype=np.float32)
    freqs = np.asarray(freqs, dtype=np.float32)
    grid = [int(v) for v in np.asarray(grid_sizes).reshape(-1)[:3]]
    cur = int(np.asarray(current_start))
    assert grid[0] * grid[1] * grid[2] == S and cur == CUR_START

    frame_seqlen = grid[1] * grid[2]
    start_frame = cur // frame_seqlen
    ang = _build_angles(freqs, grid, start_frame)

    xT = np.ascontiguousarray(
        x.reshape(S, DIM).T.reshape(NH, 128, S), dtype=np.float32)
    w_T = {n: np.ascontiguousarray(np.asarray(w, np.float32).T.reshape(NH, 128, DIM))
           for n, w in (("q", wq), ("k", wk), ("v", wv))}

    trace = _timings is not None

    # ---------------- phase 1 ----------------
    p1 = _get_program("p1")
    in_maps = []
    for c in range(N_CORES):
        rs = slice(c * R, (c + 1) * R)
        in_maps.append({
            "xT": np.ascontiguousarray(xT[:, :, rs]),
            "wqT": w_T["q"], "wkT": w_T["k"], "wvT": w_T["v"],
            "bq": np.asarray(bq, np.float32), "bk": np.asarray(bk, np.float32),
            "bv": np.asarray(bv, np.float32),
            "gq": np.asarray(gq, np.float32), "gk": np.asarray(gk, np.float32),
            "ang": np.ascontiguousarray(ang[rs]),
        })
    r1 = _run(p1, in_maps, trace=trace)
    q_new = np.concatenate([r1.results[c]["q_out"] for c in range(N_CORES)])
    k_new = np.concatenate([r1.results[c]["k_out"] for c in range(N_CORES)])
    v_new = np.concatenate([r1.results[c]["v_out"] for c in range(N_CORES)])

    # ---------------- host reshuffle for phase 2 ----------------
    k_win = np.concatenate([
        np.asarray(kv_cache_k, np.float32)[0, :cur].reshape(cur, DIM),
        k_new]).reshape(WIN, NH, HD)
    v_win = np.concatenate([
        np.asarray(kv_cache_v, np.float32)[0, :cur].reshape(cur, DIM),
        v_new]).reshape(WIN, NH, HD)
    kT_heads = np.ascontiguousarray(k_win.transpose(1, 2, 0))   # [NH,128,WIN]
    v_pad = np.zeros((NH, N_KB * KB, HD), np.float32)
    v_pad[:, :WIN] = v_win.transpose(1, 0, 2)
    qT_heads = np.ascontiguousarray(
        q_new.reshape(S, NH, HD).transpose(1, 2, 0))            # [NH,128,S]

    p2 = _get_program("p2")
    in_maps = []
    for c in range(N_CORES):
        heads = [(3 * c + u) // 2 for u in range(UNITS)]
        halves = [(3 * c + u) % 2 for u in range(UNITS)]
        in_maps.append({
            "kT": np.ascontiguousarray(kT_heads[heads]),
            "v": np.ascontiguousarray(v_pad[heads]),
            "ones": np.ones((128, 1), np.float32),
            "qT": np.ascontiguousarray(np.stack(
                [qT_heads[h, :, hf * HH:(hf + 1) * HH]
                 for h, hf in zip(heads, halves)])),
        })
    r2 = _run(p2, in_maps, trace=trace)

    aoT = np.empty((NH, HD, S), np.float32)
    for c in range(N_CORES):
        for u in range(UNITS):
            h, hf = (3 * c + u) // 2, (3 * c + u) % 2
            aoT[h, :, hf * HH:(hf + 1) * HH] = r2.results[c]["aoT"][u]

    # ---------------- phase 3 ----------------
    p3 = _get_program("p3")
    woT = np.ascontiguousarray(np.asarray(wo, np.float32).T.reshape(NH, 128, DIM))
    in_maps = []
    for c in range(N_CORES):
        rs = slice(c * R, (c + 1) * R)
        in_maps.append({
            "aT": np.ascontiguousarray(aoT[:, :, rs]),
            "woT": woT,
            "bo": np.asarray(bo, np.float32),
        })
    r3 = _run(p3, in_maps, trace=trace)
    out = np.concatenate([r3.results[c]["out"] for c in range(N_CORES)])

    if _timings is not None:
        for name, r in (("p1", r1), ("p2", r2), ("p3", r3)):
            _timings[name] = r.exec_time_ns
    return out.reshape(1, S, DIM)

